# revision 6
# baseline (speedup 1.0000x reference)
"""MoE routed-MLP (GPTNeoX) Trainium2 kernel — 8-way F-split expert parallel.

Every core processes a 512-wide slice of the F dimension for ALL 8 experts:

    phase1: h_e[f, c] = gelu( sum_k w1[e][k, fslice_i] * x_e[k, c] + b1 )
    phase2: y_e[h, c] += sum_f w2[e][fslice_i, h] * h_e[f, c]   (partial)

The host routes tokens (top-2), packs per-expert token batches, and sums the
8 per-core partial outputs (+ b2, + combine weights). Per-core compute is
64*sum(caps) PE cycles regardless of routing imbalance — strictly better
than padding every expert to the max token count.

All matmuls are bf16 (fp32r runs at the same 1 cycle/row rate, so bf16 only
halves DMA); partial outputs are fp16. Measured rel err ~3e-3 vs the 2e-2
gate.
"""

import numpy as np
import ml_dtypes

import concourse.bass as bass  # noqa: F401
import concourse.mybir as mybir
import concourse.tile as tile
from concourse import bacc
from concourse.bass_utils import run_bass_kernel_spmd

H = 1024
F = 4096
E = 8
NCORES = 8
P = 128
KO = H // P          # 8 k-tiles for the H contraction (phase 1)
FSL = F // NCORES    # 512 f-channels per core per expert
FT = FSL // P        # 4 f-tiles (phase-1 outputs / phase-2 contraction)
HO = H // P          # 8 h-tiles (phase-2 outputs)

BF16 = ml_dtypes.bfloat16

_nc_cache = {}
_wt_cache = {}


def _chunks(cap):
    """Split [0, cap) into <=512-wide chunks, widths multiple of 8."""
    n = (cap + 511) // 512
    base = cap // n // 8 * 8
    widths = [base] * n
    rem = cap - base * n
    i = 0
    while rem > 0:
        widths[i] += 8
        rem -= 8
        i = (i + 1) % n
    out, off = [], 0
    for w in widths:
        out.append((off, w))
        off += w
    return out


def _build(caps):
    f32 = mybir.dt.float32
    bf = mybir.dt.bfloat16
    f16 = mybir.dt.float16

    nc = bacc.Bacc("TRN2", target_bir_lowering=False, debug=False)
    xs_d = [
        nc.dram_tensor(f"x{j}", [P, KO, caps[j]], bf, kind="ExternalInput").ap()
        for j in range(E)
    ]
    w1_d = [
        nc.dram_tensor(f"w1_{j}", [P, KO, FSL], bf, kind="ExternalInput").ap()
        for j in range(E)
    ]
    w2_d = [
        nc.dram_tensor(f"w2_{j}", [P, FT, H], bf, kind="ExternalInput").ap()
        for j in range(E)
    ]
    b1_d = nc.dram_tensor("b1", [P, E, FT], f32, kind="ExternalInput").ap()
    y_d = [
        nc.dram_tensor(f"y{j}", [P, HO, caps[j]], f16, kind="ExternalOutput").ap()
        for j in range(E)
    ]
    maxcap = max(caps)

    with tile.TileContext(nc) as tc:
        with (
            tc.tile_pool(name="const", bufs=1) as constp,
            tc.tile_pool(name="xp", bufs=3) as xp,
            tc.tile_pool(name="hp", bufs=2) as hp,
            tc.tile_pool(name="w1p", bufs=3) as w1p,
            tc.tile_pool(name="w2p", bufs=3) as w2p,
            tc.tile_pool(name="yp", bufs=2) as yp,
            tc.tile_pool(name="ps1", bufs=2, space="PSUM") as ps1,
            tc.tile_pool(name="ps2", bufs=2, space="PSUM") as ps2,
        ):
            b1sb = constp.tile([P, E, FT], f32)
            nc.sync.dma_start(b1sb[:], b1_d[:])

            # PE warmup: ~4.5us of dummy matmuls during the initial DMA wait
            # pre-triggers the HAM un-throttle (2.4 GHz from the first real MM)
            warm = constp.tile([P, 512], bf)
            nc.vector.memset(warm[:], 0)
            wps = ps1.tile([P, 512], f32, tag="warm", bufs=1)
            for _ in range(10):
                nc.tensor.matmul(wps[:], warm[:, :P], warm[:], start=True, stop=True)

            xs, hs, w1s, w2s = {}, {}, {}, {}

            def load_p1(j):
                # fine-grained loads ordered by first need: x chunk0, w1 f0,
                # then the rest — lets expert 0's first chain start early
                xs[j] = xp.tile([P, KO, maxcap], bf, tag="x", name=f"x{j}sb")
                w1s[j] = w1p.tile([P, KO, FSL], bf, tag="w1", name=f"w1sb{j}")
                ch = _chunks(caps[j])
                c0, cw = ch[0]
                nc.sync.dma_start(xs[j][:, :, c0 : c0 + cw], xs_d[j][:, :, c0 : c0 + cw])
                nc.sync.dma_start(w1s[j][:, :, 0:P], w1_d[j][:, :, 0:P])
                for c0, cw in ch[1:]:
                    nc.sync.dma_start(
                        xs[j][:, :, c0 : c0 + cw], xs_d[j][:, :, c0 : c0 + cw]
                    )
                for f in range(1, FT):
                    nc.sync.dma_start(
                        w1s[j][:, :, f * P : (f + 1) * P],
                        w1_d[j][:, :, f * P : (f + 1) * P],
                    )

            def load_p2(j):
                w2s[j] = w2p.tile([P, FT, H], bf, tag="w2", name=f"w2sb{j}")
                nc.sync.dma_start(w2s[j][:], w2_d[j][:])

            def p1(j):
                cap = caps[j]
                hs[j] = hp.tile([P, FT, maxcap], bf, tag="h", name=f"hsb{j}")
                for f in range(FT):
                    for c0, cw in _chunks(cap):
                        ps = ps1.tile([P, 512], f32, tag="ps1")
                        for ko in range(KO):
                            nc.tensor.matmul(
                                ps[:, :cw],
                                w1s[j][:, ko, f * P : (f + 1) * P],
                                xs[j][:, ko, c0 : c0 + cw],
                                start=(ko == 0),
                                stop=(ko == KO - 1),
                            )
                        nc.scalar.activation(
                            hs[j][:, f, c0 : c0 + cw],
                            ps[:, :cw],
                            mybir.ActivationFunctionType.Gelu,
                            bias=b1sb[:, j, f : f + 1],
                        )

            def p2(j):
                cap = caps[j]
                ysb = yp.tile([P, HO, maxcap], f16, tag="y")
                for c0, cw in _chunks(cap):
                    for ho in range(HO):
                        ps = ps2.tile([P, 512], f32, tag="ps2")
                        for ko in range(FT):
                            nc.tensor.matmul(
                                ps[:, :cw],
                                w2s[j][:, ko, ho * P : (ho + 1) * P],
                                hs[j][:, ko, c0 : c0 + cw],
                                start=(ko == 0),
                                stop=(ko == FT - 1),
                            )
                        # alternate copy engines so the drain isn't DVE-serial
                        if ho % 2 == 0:
                            nc.vector.tensor_scalar_mul(
                                ysb[:, ho, c0 : c0 + cw], ps[:, :cw], 1.0
                            )
                        else:
                            nc.scalar.copy(ysb[:, ho, c0 : c0 + cw], ps[:, :cw])
                        # write back immediately — keeps the tail short
                        nc.sync.dma_start(
                            y_d[j][:, ho, c0 : c0 + cw], ysb[:, ho, c0 : c0 + cw]
                        )

            load_p1(0)
            load_p1(1)
            load_p2(0)
            p1(0)
            for j in range(1, E):
                if j + 1 < E:
                    load_p1(j + 1)
                load_p2(j)
                p1(j)
                p2(j - 1)
            p2(E - 1)
    nc.compile()
    return nc


def _get_nc(caps):
    key = tuple(caps)
    if key not in _nc_cache:
        _nc_cache[key] = _build(caps)
    return _nc_cache[key]


def _route(x, router_w):
    """Top-2 routing identical (up to fp noise far below the tie margin)
    to jax.lax.top_k + softmax in the reference."""
    n = x.shape[0]
    logits = x.astype(np.float64) @ router_w.astype(np.float64)
    r = np.arange(n)
    i1 = np.argmax(logits, 1)
    masked = logits.copy()
    masked[r, i1] = -np.inf
    i2 = np.argmax(masked, 1)
    tl = np.stack([logits[r, i1], logits[r, i2]], 1).astype(np.float32)
    e = np.exp(tl - tl.max(1, keepdims=True))
    s = (e / e.sum(1, keepdims=True)).astype(np.float32)
    return np.stack([i1, i2], 1), s


def _pack_weights(w1, b1, w2):
    """Per-core, per-slot weight tensors (bf16) — cached across calls."""
    key = (id(w1), id(w2))
    if key in _wt_cache:
        return _wt_cache[key]
    w1b = w1.astype(BF16)  # [E, H, F]
    w2b = w2.astype(BF16)  # [E, F, H]
    w1_sl = []  # [core][expert] -> [P, KO, FSL]
    w2_sl = []  # [core][expert] -> [P, FT, H]
    b1_sl = []  # [core] -> [P, E_slots?, FT] built later per order
    for i in range(NCORES):
        w1_sl.append(
            [
                np.ascontiguousarray(
                    w1b[e, :, i * FSL : (i + 1) * FSL]
                    .reshape(KO, P, FSL)
                    .transpose(1, 0, 2)
                )
                for e in range(E)
            ]
        )
        w2_sl.append(
            [
                np.ascontiguousarray(
                    w2b[e, i * FSL : (i + 1) * FSL, :]
                    .reshape(FT, P, H)
                    .transpose(1, 0, 2)
                )
                for e in range(E)
            ]
        )
        b1_sl.append(
            np.ascontiguousarray(
                np.stack(
                    [
                        b1[e, i * FSL : (i + 1) * FSL].reshape(FT, P).T
                        for e in range(E)
                    ],
                    axis=1,
                ).astype(np.float32)
            )
        )  # [P, E, FT]
    _wt_cache[key] = ((w1, w2), w1_sl, w2_sl, b1_sl)  # hold refs so ids stay valid
    return _wt_cache[key]


def _prepare(inputs):
    hs_in = np.asarray(inputs["hidden_states"], np.float32)
    router_w = np.asarray(inputs["router_w"], np.float32)
    w1 = np.asarray(inputs["w1"], np.float32)
    b1 = np.asarray(inputs["b1"], np.float32)
    w2 = np.asarray(inputs["w2"], np.float32)
    b2 = np.asarray(inputs["b2"], np.float32)
    S, B, H_ = hs_in.shape
    x = hs_in.reshape(S * B, H_)

    idx2, scores = _route(x, router_w)
    tok = [np.flatnonzero((idx2 == e).any(1)) for e in range(E)]
    wts = []
    for e in range(E):
        sel = idx2[tok[e]] == e
        wts.append(
            np.where(sel[:, 0], scores[tok[e], 0], scores[tok[e], 1]).astype(
                np.float32
            )
        )

    # slot j holds expert order[j]; process big experts first
    order = sorted(range(E), key=lambda e: -len(tok[e]))
    caps = [max(64, -(-len(tok[order[j]]) // 8) * 8) for j in range(E)]

    _, w1_sl, w2_sl, b1_sl = _pack_weights(w1, b1, w2)

    # token batches (shared across cores)
    xpk = []
    for j in range(E):
        e = order[j]
        n_e = len(tok[e])
        xe = np.zeros((P, KO, caps[j]), BF16)
        xe[:, :, :n_e] = x[tok[e]].T.reshape(KO, P, n_e).transpose(1, 0, 2)
        xpk.append(xe)

    in_maps = []
    for i in range(NCORES):
        m = {"b1": np.ascontiguousarray(b1_sl[i][:, order, :])}
        for j in range(E):
            e = order[j]
            m[f"x{j}"] = xpk[j]
            m[f"w1_{j}"] = w1_sl[i][e]
            m[f"w2_{j}"] = w2_sl[i][e]
        in_maps.append(m)

    meta = dict(
        shape=(S, B, H_), tok=tok, wts=wts, order=order, caps=caps, b2=b2
    )
    return meta, in_maps


def _combine(meta, results):
    S, B, H_ = meta["shape"]
    tok, wts, order = meta["tok"], meta["wts"], meta["order"]
    b2 = meta["b2"]
    out = np.zeros((S * B, H_), np.float32)
    for j in range(E):
        e = order[j]
        n_e = len(tok[e])
        if n_e == 0:
            continue
        Y = results[0][f"y{j}"][:, :, :n_e].astype(np.float32)
        for i in range(1, NCORES):
            Y += results[i][f"y{j}"][:, :, :n_e].astype(np.float32)
        y_tok = Y.transpose(2, 1, 0).reshape(n_e, H_)
        out[tok[e]] += wts[e][:, None] * (y_tok + b2[e][None, :])
    return out.reshape(S, B, H_)


def kernel(**inputs):
    meta, in_maps = _prepare(inputs)
    nc = _get_nc(meta["caps"])
    res = run_bass_kernel_spmd(nc, in_maps, core_ids=list(range(NCORES)))
    return _combine(meta, res.results)


# revision 7
# speedup vs baseline: 1.1522x; 1.1522x over previous
"""MoE routed-MLP (GPTNeoX) Trainium2 kernel — 8-way F-split expert parallel.

Every core processes a 512-wide slice of the F dimension for ALL 8 experts:

    phase1: h_e[f, c] = gelu( sum_k w1[e][k, fslice_i] * x_e[k, c] + b1 )
    phase2: y_e[h, c] += sum_f w2[e][fslice_i, h] * h_e[f, c]   (partial)

The host routes tokens (top-2), packs per-expert token batches, and sums the
8 per-core partial outputs (+ b2, + combine weights). Per-core compute is
64*sum(caps) PE cycles regardless of routing imbalance — strictly better
than padding every expert to the max token count.

All matmuls are bf16 (fp32r runs at the same 1 cycle/row rate, so bf16 only
halves DMA); partial outputs are fp16. Measured rel err ~3e-3 vs the 2e-2
gate.
"""

import numpy as np
import ml_dtypes

import concourse.bass as bass  # noqa: F401
import concourse.mybir as mybir
import concourse.tile as tile
from concourse import bacc
from concourse.bass_utils import run_bass_kernel_spmd

H = 1024
F = 4096
E = 8
NCORES = 8
P = 128
KO = H // P          # 8 k-tiles for the H contraction (phase 1)
FSL = F // NCORES    # 512 f-channels per core per expert
FT = FSL // P        # 4 f-tiles (phase-1 outputs / phase-2 contraction)
HO = H // P          # 8 h-tiles (phase-2 outputs)

BF16 = ml_dtypes.bfloat16

_nc_cache = {}
_wt_cache = {}


def _chunks(cap):
    """Split [0, cap) into <=512-wide chunks, widths multiple of 8."""
    n = (cap + 511) // 512
    base = cap // n // 8 * 8
    widths = [base] * n
    rem = cap - base * n
    i = 0
    while rem > 0:
        widths[i] += 8
        rem -= 8
        i = (i + 1) % n
    out, off = [], 0
    for w in widths:
        out.append((off, w))
        off += w
    return out


def _build(caps):
    f32 = mybir.dt.float32
    bf = mybir.dt.bfloat16
    f16 = mybir.dt.float16

    nc = bacc.Bacc("TRN2", target_bir_lowering=False, debug=False)
    xs_d = [
        nc.dram_tensor(f"x{j}", [P, KO, caps[j]], bf, kind="ExternalInput").ap()
        for j in range(E)
    ]
    w1_d = [
        nc.dram_tensor(f"w1_{j}", [P, KO, FSL], bf, kind="ExternalInput").ap()
        for j in range(E)
    ]
    w2_d = [
        nc.dram_tensor(f"w2_{j}", [P, FT, H], bf, kind="ExternalInput").ap()
        for j in range(E)
    ]
    b1_d = nc.dram_tensor("b1", [P, E, FT], f32, kind="ExternalInput").ap()
    y_d = [
        nc.dram_tensor(f"y{j}", [P, HO, caps[j]], f16, kind="ExternalOutput").ap()
        for j in range(E)
    ]
    maxcap = max(caps)

    with tile.TileContext(nc) as tc:
        with (
            tc.tile_pool(name="const", bufs=1) as constp,
            tc.tile_pool(name="xp", bufs=3) as xp,
            tc.tile_pool(name="hp", bufs=2) as hp,
            tc.tile_pool(name="w1p", bufs=3) as w1p,
            tc.tile_pool(name="w2p", bufs=3) as w2p,
            tc.tile_pool(name="yp", bufs=2) as yp,
            tc.tile_pool(name="ps1", bufs=2, space="PSUM") as ps1,
            tc.tile_pool(name="ps2", bufs=2, space="PSUM") as ps2,
        ):
            b1sb = constp.tile([P, E, FT], f32)
            nc.sync.dma_start(b1sb[:], b1_d[:])

            # PE warmup: ~4.5us of dummy matmuls during the initial DMA wait
            # pre-triggers the HAM un-throttle (2.4 GHz from the first real MM)
            warm = constp.tile([P, 512], bf)
            nc.vector.memset(warm[:], 0)
            wps = ps1.tile([P, 512], f32, tag="warm", bufs=1)
            for _ in range(10):
                nc.tensor.matmul(wps[:], warm[:, :P], warm[:], start=True, stop=True)

            xs, hs, w1s, w2s = {}, {}, {}, {}

            def load_p1(j):
                # fine-grained loads ordered by first need: x chunk0, w1 f0,
                # then the rest — lets expert 0's first chain start early
                xs[j] = xp.tile([P, KO, maxcap], bf, tag="x", name=f"x{j}sb")
                w1s[j] = w1p.tile([P, KO, FSL], bf, tag="w1", name=f"w1sb{j}")
                ch = _chunks(caps[j])
                c0, cw = ch[0]
                nc.sync.dma_start(xs[j][:, :, c0 : c0 + cw], xs_d[j][:, :, c0 : c0 + cw])
                nc.sync.dma_start(w1s[j][:, :, 0:P], w1_d[j][:, :, 0:P])
                for c0, cw in ch[1:]:
                    nc.sync.dma_start(
                        xs[j][:, :, c0 : c0 + cw], xs_d[j][:, :, c0 : c0 + cw]
                    )
                for f in range(1, FT):
                    nc.sync.dma_start(
                        w1s[j][:, :, f * P : (f + 1) * P],
                        w1_d[j][:, :, f * P : (f + 1) * P],
                    )

            def load_p2(j):
                w2s[j] = w2p.tile([P, FT, H], bf, tag="w2", name=f"w2sb{j}")
                nc.sync.dma_start(w2s[j][:], w2_d[j][:])

            def p1(j):
                cap = caps[j]
                hs[j] = hp.tile([P, FT, maxcap], bf, tag="h", name=f"hsb{j}")
                for f in range(FT):
                    for c0, cw in _chunks(cap):
                        ps = ps1.tile([P, 512], f32, tag="ps1")
                        for ko in range(KO):
                            nc.tensor.matmul(
                                ps[:, :cw],
                                w1s[j][:, ko, f * P : (f + 1) * P],
                                xs[j][:, ko, c0 : c0 + cw],
                                start=(ko == 0),
                                stop=(ko == KO - 1),
                            )
                        nc.scalar.activation(
                            hs[j][:, f, c0 : c0 + cw],
                            ps[:, :cw],
                            mybir.ActivationFunctionType.Gelu,
                            bias=b1sb[:, j, f : f + 1],
                        )

            def p2(j):
                cap = caps[j]
                ysb = yp.tile([P, HO, maxcap], f16, tag="y")
                for c0, cw in _chunks(cap):
                    for ho in range(HO):
                        ps = ps2.tile([P, 512], f32, tag="ps2")
                        for ko in range(FT):
                            nc.tensor.matmul(
                                ps[:, :cw],
                                w2s[j][:, ko, ho * P : (ho + 1) * P],
                                hs[j][:, ko, c0 : c0 + cw],
                                start=(ko == 0),
                                stop=(ko == FT - 1),
                            )
                        nc.vector.tensor_scalar_mul(
                            ysb[:, ho, c0 : c0 + cw], ps[:, :cw], 1.0
                        )
                        # write back immediately on the (otherwise idle)
                        # GpSimd DMA queue so input prefetch FIFO can't
                        # head-of-line block the ysb frees
                        nc.gpsimd.dma_start(
                            y_d[j][:, ho, c0 : c0 + cw], ysb[:, ho, c0 : c0 + cw]
                        )

            load_p1(0)
            load_p1(1)
            load_p2(0)
            p1(0)
            for j in range(1, E):
                if j + 1 < E:
                    load_p1(j + 1)
                load_p2(j)
                p1(j)
                p2(j - 1)
            p2(E - 1)
    nc.compile()
    return nc


def _get_nc(caps):
    key = tuple(caps)
    if key not in _nc_cache:
        _nc_cache[key] = _build(caps)
    return _nc_cache[key]


def _route(x, router_w):
    """Top-2 routing identical (up to fp noise far below the tie margin)
    to jax.lax.top_k + softmax in the reference."""
    n = x.shape[0]
    logits = x.astype(np.float64) @ router_w.astype(np.float64)
    r = np.arange(n)
    i1 = np.argmax(logits, 1)
    masked = logits.copy()
    masked[r, i1] = -np.inf
    i2 = np.argmax(masked, 1)
    tl = np.stack([logits[r, i1], logits[r, i2]], 1).astype(np.float32)
    e = np.exp(tl - tl.max(1, keepdims=True))
    s = (e / e.sum(1, keepdims=True)).astype(np.float32)
    return np.stack([i1, i2], 1), s


def _pack_weights(w1, b1, w2):
    """Per-core, per-slot weight tensors (bf16) — cached across calls."""
    key = (id(w1), id(w2))
    if key in _wt_cache:
        return _wt_cache[key]
    w1b = w1.astype(BF16)  # [E, H, F]
    w2b = w2.astype(BF16)  # [E, F, H]
    w1_sl = []  # [core][expert] -> [P, KO, FSL]
    w2_sl = []  # [core][expert] -> [P, FT, H]
    b1_sl = []  # [core] -> [P, E_slots?, FT] built later per order
    for i in range(NCORES):
        w1_sl.append(
            [
                np.ascontiguousarray(
                    w1b[e, :, i * FSL : (i + 1) * FSL]
                    .reshape(KO, P, FSL)
                    .transpose(1, 0, 2)
                )
                for e in range(E)
            ]
        )
        w2_sl.append(
            [
                np.ascontiguousarray(
                    w2b[e, i * FSL : (i + 1) * FSL, :]
                    .reshape(FT, P, H)
                    .transpose(1, 0, 2)
                )
                for e in range(E)
            ]
        )
        b1_sl.append(
            np.ascontiguousarray(
                np.stack(
                    [
                        b1[e, i * FSL : (i + 1) * FSL].reshape(FT, P).T
                        for e in range(E)
                    ],
                    axis=1,
                ).astype(np.float32)
            )
        )  # [P, E, FT]
    _wt_cache[key] = ((w1, w2), w1_sl, w2_sl, b1_sl)  # hold refs so ids stay valid
    return _wt_cache[key]


def _prepare(inputs):
    hs_in = np.asarray(inputs["hidden_states"], np.float32)
    router_w = np.asarray(inputs["router_w"], np.float32)
    w1 = np.asarray(inputs["w1"], np.float32)
    b1 = np.asarray(inputs["b1"], np.float32)
    w2 = np.asarray(inputs["w2"], np.float32)
    b2 = np.asarray(inputs["b2"], np.float32)
    S, B, H_ = hs_in.shape
    x = hs_in.reshape(S * B, H_)

    idx2, scores = _route(x, router_w)
    tok = [np.flatnonzero((idx2 == e).any(1)) for e in range(E)]
    wts = []
    for e in range(E):
        sel = idx2[tok[e]] == e
        wts.append(
            np.where(sel[:, 0], scores[tok[e], 0], scores[tok[e], 1]).astype(
                np.float32
            )
        )

    # slot j holds expert order[j]; process big experts first
    order = sorted(range(E), key=lambda e: -len(tok[e]))
    caps = [max(64, -(-len(tok[order[j]]) // 8) * 8) for j in range(E)]

    _, w1_sl, w2_sl, b1_sl = _pack_weights(w1, b1, w2)

    # token batches (shared across cores)
    xpk = []
    for j in range(E):
        e = order[j]
        n_e = len(tok[e])
        xe = np.zeros((P, KO, caps[j]), BF16)
        xe[:, :, :n_e] = x[tok[e]].T.reshape(KO, P, n_e).transpose(1, 0, 2)
        xpk.append(xe)

    in_maps = []
    for i in range(NCORES):
        m = {"b1": np.ascontiguousarray(b1_sl[i][:, order, :])}
        for j in range(E):
            e = order[j]
            m[f"x{j}"] = xpk[j]
            m[f"w1_{j}"] = w1_sl[i][e]
            m[f"w2_{j}"] = w2_sl[i][e]
        in_maps.append(m)

    meta = dict(
        shape=(S, B, H_), tok=tok, wts=wts, order=order, caps=caps, b2=b2
    )
    return meta, in_maps


def _combine(meta, results):
    S, B, H_ = meta["shape"]
    tok, wts, order = meta["tok"], meta["wts"], meta["order"]
    b2 = meta["b2"]
    out = np.zeros((S * B, H_), np.float32)
    for j in range(E):
        e = order[j]
        n_e = len(tok[e])
        if n_e == 0:
            continue
        Y = results[0][f"y{j}"][:, :, :n_e].astype(np.float32)
        for i in range(1, NCORES):
            Y += results[i][f"y{j}"][:, :, :n_e].astype(np.float32)
        y_tok = Y.transpose(2, 1, 0).reshape(n_e, H_)
        out[tok[e]] += wts[e][:, None] * (y_tok + b2[e][None, :])
    return out.reshape(S, B, H_)


def kernel(**inputs):
    meta, in_maps = _prepare(inputs)
    nc = _get_nc(meta["caps"])
    res = run_bass_kernel_spmd(nc, in_maps, core_ids=list(range(NCORES)))
    return _combine(meta, res.results)
